# revision 7
# baseline (speedup 1.0000x reference)
"""Trainium2 Bass kernel for causal multi-head attention (B=4, T=2048, D=1024, H=16).

Sharding: 8 cores = 4 batches x 2 head-groups (8 heads each).
Per core pipeline (Tile framework, single SPMD program):
  phase 1(j): Q/K projections into transposed per-head-pair layout QT/KT [128=2*64, T],
           V projection into [t, 8*65] layout (65th col per head = ones, for rowsums)
  phase 2(j): per (q-range of 512, head-pair): causal flash attention in transposed
           layout: ST[k,q] = KT-slice^T @ QT-slice (row-packed pair of matmuls),
           PT = exp(ST) (ACT), causal triangle mask on diagonal 128-col strips (DVE),
           OT[hd+1, q] += [V|1]^T @ PT (bf16), normalize by approx-reciprocal rowsum.
  phase 3(j): output projection YT[dout, t] = Wo_gT^T @ OT, DMA'd straight from PSUM.
Phase 1(j+1) matmul chunks are emitted interleaved into phase 2(j) so the PE fills
its exp-wait gaps with projection work (phase 2 is ACT-bound; phases 1/3 PE-bound).
No collective: each core emits its partial YT [D, T]; the host adds the two partial
sums of each batch pair and adds the output bias.
"""

import numpy as np

B, T, D, H, HD = 4, 2048, 1024, 16, 64
NCORES = 8
NP = 4          # head pairs per core
NJ = 4          # q-ranges of 512
QW = 512
TB = T // 128   # 16

_CACHE = {}


def _build_nc():
    import concourse.mybir as mybir
    import concourse.tile as tile
    from concourse import bacc

    F32 = mybir.dt.float32
    BF16 = mybir.dt.bfloat16
    AF = mybir.ActivationFunctionType

    nc = bacc.Bacc(None, target_bir_lowering=False)
    xt_d = nc.declare_dram_parameter("xt", [D, T], BF16, isOutput=False)
    wq_d = nc.declare_dram_parameter("wq", [D, 512], BF16, isOutput=False)
    wk_d = nc.declare_dram_parameter("wk", [D, 512], BF16, isOutput=False)
    wv_d = nc.declare_dram_parameter("wv", [D, 512], BF16, isOutput=False)
    wo_d = nc.declare_dram_parameter("wo", [512, D], BF16, isOutput=False)
    mask_d = nc.declare_dram_parameter("mask", [128, 128], BF16, isOutput=False)
    yt_d = nc.declare_dram_parameter("yt", [D, T], F32, isOutput=True)

    with tile.TileContext(nc) as tc:
        with (
            tc.tile_pool(name="persist", bufs=1) as pers,
            tc.tile_pool(name="work", bufs=1) as work,
            tc.tile_pool(name="psum", bufs=1, space="PSUM") as psum,
        ):
            qt = pers.tile([128, NP, T], BF16)
            kt = pers.tile([128, NP, T], BF16)
            v = pers.tile([128, TB, 8 * 65], BF16)
            ot = pers.tile([128, NP, T], BF16)
            m0 = pers.tile([128, 128], BF16)
            wo = pers.tile([128, 4, D], BF16)
            wq = pers.tile([128, 8, 512], BF16)
            wk = pers.tile([128, 8, 512], BF16)
            wv = pers.tile([128, 8, 512], BF16)

            xs_tiles = {}

            def load_xs(j):
                for c in range(8):
                    t = work.tile([128, QW], BF16, tag="xs", bufs=16)
                    nc.sync.dma_start(
                        out=t[:], in_=xt_d[c * 128:(c + 1) * 128, j * QW:(j + 1) * QW]
                    )
                    xs_tiles[(j, c)] = t

            # startup DMAs, ordered so phase 1(0) can begin as early as possible
            nc.sync.dma_start(out=m0[:], in_=mask_d[:])
            for c in range(8):
                nc.sync.dma_start(out=wq[:, c, :], in_=wq_d[c * 128:(c + 1) * 128, :])
            load_xs(0)
            for c in range(8):
                nc.sync.dma_start(out=wk[:, c, :], in_=wk_d[c * 128:(c + 1) * 128, :])
            for c in range(8):
                nc.sync.dma_start(out=wv[:, c, :], in_=wv_d[c * 128:(c + 1) * 128, :])
            nc.sync.dma_start(out=wo[:], in_=wo_d.rearrange("(c p) n -> p c n", p=128))

            def emit_p1_qk(j, p, w_sb, dst):
                acc = psum.tile([128, QW], F32, tag="small", bufs=2)
                for c in range(8):
                    nc.tensor.matmul(
                        acc[:],
                        w_sb[:, c, p * 128:(p + 1) * 128],
                        xs_tiles[(j, c)][:],
                        start=(c == 0),
                        stop=(c == 7),
                    )
                nc.vector.tensor_copy(dst[:, p, j * QW:(j + 1) * QW], acc[:])

            def emit_p1_v(j, sub):
                i = 4 * j + sub
                acc = psum.tile([128, QW], F32, tag="small", bufs=2)
                for c in range(8):
                    nc.tensor.matmul(
                        acc[:],
                        xs_tiles[(j, c)][:, sub * 128:(sub + 1) * 128],
                        wv[:, c, :],
                        start=(c == 0),
                        stop=(c == 7),
                    )
                vblk = v[:, i, :].rearrange("p (h c) -> p h c", c=65)
                nc.vector.tensor_copy(
                    vblk[:, :, 0:64],
                    acc[:].rearrange("p (h c) -> p h c", c=64),
                )
                nc.gpsimd.memset(vblk[:, :, 64:65], 1.0)

            def phase1_chunks(j):
                ch = []
                for p in range(NP):
                    ch.append(lambda p=p: emit_p1_qk(j, p, wq, qt))
                for p in range(NP):
                    ch.append(lambda p=p: emit_p1_qk(j, p, wk, kt))
                for sub in range(4):
                    ch.append(lambda sub=sub: emit_p1_v(j, sub))
                return ch

            # phase 1(0) up front
            for chunk in phase1_chunks(0):
                chunk()

            for j in range(NJ):
                jr = slice(j * QW, (j + 1) * QW)
                if j + 1 < NJ:
                    load_xs(j + 1)
                    chunks = phase1_chunks(j + 1)
                else:
                    chunks = []
                nkb = 4 * j + 4
                slots = NP * nkb
                emitted = 0
                slot = 0

                # ---------------- phase 2(j) with phase 1(j+1) interleaved ----------
                for p in range(NP):
                    hA, hB = 2 * p, 2 * p + 1
                    o_A = psum.tile([65, QW], F32, tag="o", bufs=2)
                    o_B = psum.tile([65, QW], F32, tag="o", bufs=2)
                    for kb in range(nkb):
                        o = kb - 4 * j  # diagonal offset; < 0 means full block
                        lo = 128 * o if o > 0 else 0
                        st = psum.tile([128, 1024], F32, tag="st", bufs=2)
                        kcols = slice(kb * 128, (kb + 1) * 128)
                        qcols = slice(j * QW + lo, (j + 1) * QW)
                        nc.tensor.matmul(
                            st[:, lo:QW],
                            kt[0:64, p, kcols],
                            qt[0:64, p, qcols],
                            start=True, stop=True, tile_position=(0, 0),
                        )
                        nc.tensor.matmul(
                            st[:, QW + lo:2 * QW],
                            kt[64:128, p, kcols],
                            qt[64:128, p, qcols],
                            start=True, stop=True, tile_position=(64, 0),
                        )
                        pt = work.tile([128, 1024], BF16, tag="pt", bufs=3)
                        nc.scalar.activation(
                            pt[:].rearrange("p (h q) -> p h q", h=2)[:, :, lo:QW],
                            st[:].rearrange("p (h q) -> p h q", h=2)[:, :, lo:QW],
                            AF.Exp,
                        )
                        if o >= 0:
                            # only the leading 128-col strip of the valid range
                            # holds the causal triangle
                            nc.vector.tensor_mul(
                                pt[:, lo:lo + 128], pt[:, lo:lo + 128], m0[:]
                            )
                            nc.vector.tensor_mul(
                                pt[:, QW + lo:QW + lo + 128],
                                pt[:, QW + lo:QW + lo + 128],
                                m0[:],
                            )
                        nc.tensor.matmul(
                            o_A[:, lo:QW],
                            v[:, kb, hA * 65:(hA + 1) * 65],
                            pt[:, lo:QW],
                            start=(kb == 0), stop=(kb == nkb - 1),
                        )
                        nc.tensor.matmul(
                            o_B[:, lo:QW],
                            v[:, kb, hB * 65:(hB + 1) * 65],
                            pt[:, QW + lo:2 * QW],
                            start=(kb == 0), stop=(kb == nkb - 1),
                        )
                        slot += 1
                        while (
                            emitted < len(chunks)
                            and slot >= (emitted + 1) * slots // (len(chunks) + 1)
                        ):
                            chunks[emitted]()
                            emitted += 1
                    # normalize: ot[:, p, jr] = o / rowsum (stage through SBUF;
                    # custom-DVE recip from PSUM misbehaves under load)
                    ocp = work.tile([65, 1024], F32, tag="ocp", bufs=3)
                    nc.vector.tensor_copy(ocp[:, 0:QW], o_A[:])
                    nc.vector.tensor_copy(ocp[:, QW:1024], o_B[:])
                    rec = work.tile([1, 1024], F32, tag="rec", bufs=2)
                    nc.vector.reciprocal(rec[:], ocp[64:65, :])
                    bc = work.tile([64, 1024], F32, tag="bc", bufs=2)
                    nc.gpsimd.partition_broadcast(bc[:, 0:QW], rec[:, 0:QW], channels=64)
                    nc.gpsimd.partition_broadcast(bc[:, QW:1024], rec[:, QW:1024], channels=64)
                    nc.vector.tensor_mul(ot[0:64, p, jr], ocp[0:64, 0:QW], bc[:, 0:QW])
                    nc.vector.tensor_mul(ot[64:128, p, jr], ocp[0:64, QW:1024], bc[:, QW:1024])
                while emitted < len(chunks):
                    chunks[emitted]()
                    emitted += 1

                # ---------------- phase 3(j): output projection, DMA from PSUM ------
                for n in range(8):
                    yps = psum.tile([128, QW], F32, tag="small", bufs=2)
                    for c in range(4):
                        nc.tensor.matmul(
                            yps[:],
                            wo[:, c, n * 128:(n + 1) * 128],
                            ot[:, c, jr],
                            start=(c == 0), stop=(c == 3),
                        )
                    ysb = work.tile([128, QW], F32, tag="ysb", bufs=3)
                    nc.vector.tensor_copy(ysb[:], yps[:])
                    nc.sync.dma_start(
                        out=yt_d[n * 128:(n + 1) * 128, jr], in_=ysb[:]
                    )

    nc.finalize()
    return nc


def _prep_inputs(x, Wq, Wk, Wv, Wo, bo):
    """Build the 8 per-core input maps (host-side layout prep only)."""
    import ml_dtypes

    scale = 1.0 / np.sqrt(np.float32(HD))
    kr = np.arange(128, dtype=np.float32)[:, None]
    qc = np.arange(128, dtype=np.float32)[None, :]
    m0 = (qc >= kr).astype(ml_dtypes.bfloat16)

    xts = [np.ascontiguousarray(x[b].T).astype(ml_dtypes.bfloat16) for b in range(B)]
    in_maps = []
    for c in range(NCORES):
        b, g = c // 2, c % 2
        hs = slice(g * 8, (g + 1) * 8)
        wqc = np.ascontiguousarray(Wq[hs].reshape(512, D).T * scale).astype(ml_dtypes.bfloat16)
        wkc = np.ascontiguousarray(Wk[hs].reshape(512, D).T).astype(ml_dtypes.bfloat16)
        wvc = np.ascontiguousarray(Wv[hs].reshape(512, D).T).astype(ml_dtypes.bfloat16)
        woc = np.ascontiguousarray(Wo[:, g * 512:(g + 1) * 512].T).astype(ml_dtypes.bfloat16)
        in_maps.append(
            {"xt": xts[b], "wq": wqc, "wk": wkc, "wv": wvc, "wo": woc, "mask": m0}
        )
    return in_maps


def _assemble(yts, bo):
    """Sum the per-core partial outputs of each batch pair, add bias."""
    y = np.empty((B, T, D), np.float32)
    for b in range(B):
        y[b] = (yts[2 * b] + yts[2 * b + 1]).T
    y += bo.astype(np.float32)[None, None, :]
    return y


def _run(inputs, trace=False, trace_cores=None):
    from concourse.bass_utils import run_bass_kernel_spmd

    if "nc" not in _CACHE:
        _CACHE["nc"] = _build_nc()
    nc = _CACHE["nc"]
    in_maps = _prep_inputs(
        inputs["x"], inputs["Wq"], inputs["Wk"], inputs["Wv"], inputs["Wo"], inputs["bo"]
    )
    r = run_bass_kernel_spmd(
        nc, in_maps, list(range(NCORES)), trace=trace, trace_cores=trace_cores
    )
    y = _assemble([r.results[c]["yt"] for c in range(NCORES)], inputs["bo"])
    return y, r


def kernel(**inputs):
    y, _ = _run(inputs, trace=False)
    return y
